# revision 5
# baseline (speedup 1.0000x reference)
"""Bass/Trainium2 kernel for nn_GroundingLoss (symmetric token-level InfoNCE).

Math (matches the jax reference exactly):
    sim[a,b,i,j] = sum_k x[a,i,k] * z[b,j,k]
    S[a,b]       = (1/J) * sum_j  [ sum_i softmax_i(sim[a,b,:,j]) * sim[a,b,:,j] ]
    loss         = mean( logsumexp_a(S) - diag + logsumexp_b(S) - diag )

Sharding: the batch axis of x (a) is split across the 8 cores; every core
computes S[a_local, :] against all of z.

Device layout per core (v5): partitions = (b4, j32) per (b,j)-tile (64 tiles
of 128), free = (i, a) with i major.  This puts the softmax i-reduction on
the FREE axis so the PE only does the single sim pass (the v2 layout burned
half its PE time on ones-matmul partition reductions).  Tiles run in PAIRS
(PSUM holds 2 double-buffered [128,2048]f32 groups) so every downstream op
covers 2048 elements, amortizing per-op init cycles.  Per pair:
  PE   8 matmuls (K=256 as 2 accum halves, weights reused across free halves)
  ACT  e = exp(sim - SHIFT)                (PSUM -> SBUF bf16; exp only
       exists on ACT, ~1.9us/pair: the hard floor)
  DVE  es = e * sim                        (the only vector engine allowed to
       read PSUM; ~2.3us/pair: the bottleneck)
  Pool l1 = i-halves folded for e and es   (GPSIMD/Pool engine, SBUF only)
The [128,(pt,e|es,i16,a32)] l1 tiles ship to the host as bf16 (8MB/core);
the host folds i16, divides num/den, averages over j, and runs the tiny
[256,256] logsumexp epilogue (softmax weights are shift-invariant, so no
SHIFT correction is needed).
"""

import numpy as np

N, I, J, K = 256, 32, 32, 256
NCORES = 8
NL = N // NCORES          # 32 local a's per core
AF = NL * I               # 1024 rhs cols per K-half (i, a) i-major
BJ = N * J                # 8192 (b, j) pairs
NT = BJ // 128            # 64 (b,j)-tiles of 128 partitions
NP = NT // 2              # 32 tile-pairs
SHIFT = 60.0              # exp shift: safe for |sim| up to ~130

_cached = None


def _build():
    import concourse.bacc as bacc
    import concourse.mybir as mybir
    import concourse.tile as tile

    f32 = mybir.dt.float32
    bf16 = mybir.dt.bfloat16
    AF_T = mybir.ActivationFunctionType

    nc = bacc.Bacc("TRN2", target_bir_lowering=False, debug=False)
    xt_d = nc.dram_tensor("xt", [128, 2 * AF], bf16, kind="ExternalInput").ap()
    zt_d = nc.dram_tensor("zt", [128, 2 * BJ], bf16, kind="ExternalInput").ap()
    out_d = nc.dram_tensor("out", [128, NP, 2, 2, 512], bf16, kind="ExternalOutput").ap()

    with tile.TileContext(nc) as tc:
        with (
            tc.tile_pool(name="const", bufs=1) as cpool,
            tc.tile_pool(name="psum", bufs=2, space="PSUM") as ppool,
            tc.tile_pool(name="ees", bufs=3) as epool,
            tc.tile_pool(name="l1", bufs=3) as lpool,
        ):
            bias_t = cpool.tile([128, 1], f32)
            nc.gpsimd.memset(bias_t[:], -SHIFT)
            xt = cpool.tile([128, 2 * AF], bf16)
            zt = cpool.tile([128, 2 * BJ], bf16)
            # input loads go on the ACT hwdge queue, keeping the SP queue
            # free for the steady stream of output DMAs; interleave the two
            # K-halves so early tiles are ready fast
            nc.scalar.dma_start(xt[:], xt_d[:, :])
            nq = 8
            for q in range(nq):
                for kc in range(2):
                    sl = slice(kc * BJ + q * (BJ // nq), kc * BJ + (q + 1) * (BJ // nq))
                    nc.scalar.dma_start(zt[:, sl], zt_d[:, sl])

            for pr in range(NP):
                sim = ppool.tile([128, 2, 2, 512], f32, tag="sim")  # (pt, fh, .)
                for pt in range(2):
                    t = 2 * pr + pt
                    for kc in range(2):
                        lhsT = zt[:, kc * BJ + t * 128 : kc * BJ + (t + 1) * 128]
                        for fh in range(2):
                            nc.tensor.matmul(
                                sim[:, pt, fh],
                                lhsT,
                                xt[:, kc * AF + fh * 512 : kc * AF + (fh + 1) * 512],
                                start=(kc == 0),
                                stop=(kc == 1),
                            )
                # (pt, e|es, i-half, (i16,a32)): i-halves are contiguous runs
                ees = epool.tile([128, 2, 2, 2, 512], bf16, tag="ees")
                nc.scalar.activation(
                    ees[:, :, 0], sim[:], AF_T.Exp, bias=bias_t[:], scale=1.0
                )
                nc.vector.tensor_mul(ees[:, :, 1], ees[:, :, 0], sim[:])
                # fold the two i-halves for e and es at once (i 32 -> 16);
                # the host finishes the reduction
                l1 = lpool.tile([128, 2, 2, 512], bf16, tag="l1")
                nc.gpsimd.tensor_add(l1[:], ees[:, :, :, 0, :], ees[:, :, :, 1, :])
                nc.sync.dma_start(out_d[:, pr], l1[:])
    nc.compile()
    return nc


def _prep_inputs(x, z):
    import ml_dtypes

    bf = ml_dtypes.bfloat16
    x = np.ascontiguousarray(x, dtype=np.float32).astype(bf)
    z = np.ascontiguousarray(z, dtype=np.float32).astype(bf)
    # zt[p, kc*BJ + b*J + j] = z[b, j, kc*128 + p]
    zt = z.transpose(2, 0, 1).reshape(K, BJ)
    zt = np.concatenate([zt[0:128], zt[128:256]], axis=1)
    zt = np.ascontiguousarray(zt)
    in_maps = []
    for d in range(NCORES):
        xl = x[d * NL : (d + 1) * NL]                  # [a, i, K]
        xt = xl.transpose(2, 1, 0).reshape(K, AF)      # [K, (i, a)]
        xt = np.concatenate([xt[0:128], xt[128:256]], axis=1)
        in_maps.append({"xt": np.ascontiguousarray(xt), "zt": zt})
    return in_maps


def _epilogue(results):
    S = np.empty((N, N), dtype=np.float64)
    for d in range(NCORES):
        # [p=(b4,j), t=(pr,pt), e|es, i16, a]
        arr = results[d]["out"].astype(np.float32).reshape(128, NT, 2, 16, NL)
        nd = arr.sum(axis=3)                           # [p, t, e|es, a]
        r = nd[:, :, 1] / nd[:, :, 0]                  # num/den [(b4,j), t, a]
        r = r.reshape(4, J, NT, NL).mean(axis=1).astype(np.float64)  # [b4, t, a]
        S[d * NL : (d + 1) * NL, :] = r.transpose(2, 1, 0).reshape(NL, N)
    diag = np.diagonal(S)
    m0 = S.max(axis=0)
    lx = m0 + np.log(np.exp(S - m0[None, :]).sum(axis=0)) - diag
    m1 = S.max(axis=1)
    lz = m1 + np.log(np.exp(S - m1[:, None]).sum(axis=1)) - diag
    loss = (lx + lz).mean()
    return np.asarray(loss, dtype=np.float32)


def run_on_device(x, z, trace=False):
    """Returns (loss, BassKernelResults)."""
    from concourse.bass_utils import run_bass_kernel_spmd

    global _cached
    if _cached is None:
        _cached = _build()
    nc = _cached
    in_maps = _prep_inputs(x, z)
    res = run_bass_kernel_spmd(nc, in_maps, list(range(NCORES)), trace=trace)
    return _epilogue(res.results), res


def kernel(x, z):
    loss, _ = run_on_device(x, z)
    return loss
